# revision 22
# baseline (speedup 1.0000x reference)
"""DiscreteBKI update kernel for Trainium2 (8 NeuronCores, Bass/Tile).

Pipeline (per core, x-slab of 32 planes + 1-plane halo each side):
  1. host: bucket valid points by (x-plane, y-block-of-12), precompute
     per-point (a, b) sub-indices for the on-device scatter.
  2. device: histogram scatter via one-hot matmuls accumulating in PSUM
     (exact: one-hot fp16 products accumulated in fp32).
  3. device: 3x3x3 conv as 9 banded matmuls per output plane over a
     (y%4, z) x (y//4, class) blocked layout, fused with current_map add.
  4. host: un-block the 8 output slabs (f16) into the full map (f32).

Layout: y = 4g + r;  SBUF partition p = r*32 + z;  free col f = g*21 + c.
"""

import os
import sys

import numpy as np
import ml_dtypes

for _p in (
    "/opt/trn_rl_repo",
    "/root/.axon_site/_ro/trn_rl_repo",
    "/root/.axon_site",
    "/root/.axon_site/_ro/pypackages",
):
    if os.path.isdir(_p) and _p not in sys.path:
        sys.path.append(_p)

import concourse.bacc as bacc  # noqa: E402
import concourse.mybir as mybir  # noqa: E402
import concourse.tile as tile  # noqa: E402
from concourse.bass_utils import run_bass_kernel_spmd  # noqa: E402

F16 = mybir.dt.float16
F32 = mybir.dt.float32
F8 = mybir.dt.float8e4
AF = mybir.ActivationFunctionType
ALU = mybir.AluOpType

# ---- problem geometry (hardcoded; must match the reference) ----
GX, GY, GZ, NC = 256, 256, 32, 21
MIN_B = np.array([-25.6, -25.6, -2.0], np.float32)
MAX_B = np.array([25.6, 25.6, 1.2], np.float32)
VOX = (MAX_B - MIN_B) / np.array([GX, GY, GZ], np.float32)
N_CORES = 8
XS = GX // N_CORES            # 32 x-planes owned per core
XL = XS + 2                   # 34 hist planes (with +-1 halo)
YB = 12                       # y-block per scatter bucket
NBK = 22                      # buckets per plane (21 full + 1 of width 4)
BW = 63                       # b-range per bucket (3 * 21)
SLOT = 64                     # psum cols reserved per bucket
FREE = (GY // 4) * NC         # 1344
PAD = NC                      # 21 zero cols each side of a plane tile
PLANE_F = FREE + 2 * PAD      # 1386
TPP = NBK                     # point tiles per plane (1 tile per bucket)
NPR = NBK // 2                # DoubleRow bucket-pairs per plane (11)
T_TOT = XL * TPP              # 748 point tiles per core
CHUNKS = ((0, 512), (512, 512), (1024, FREE - 1024))


def _build_masks():
    """Constant selection masks for assembling banded conv stationaries."""
    p = np.arange(128)
    r_in, z_in = p >> 5, p & 31
    m = np.arange(128)
    r_out, z_out = m >> 5, m & 31
    mask9 = np.zeros((128, 9, 128), np.float16)
    for fy in range(3):
        for fz in range(3):
            mask9[:, fy * 3 + fz, :] = (
                (r_in[:, None] - r_out[None, :] == fy - 1)
                & (z_in[:, None] - z_out[None, :] == fz - 1)
            )
    zo = np.arange(32)
    maskp = np.zeros((128, 3, 32), np.float16)
    maskm = np.zeros((128, 3, 32), np.float16)
    for fz in range(3):
        maskp[:, fz, :] = (p[:, None] < 32) & (p[:, None] - zo[None, :] == fz - 1)
        maskm[:, fz, :] = (p[:, None] >= 96) & (
            (p[:, None] - 96) - zo[None, :] == fz - 1
        )
    return (
        mask9.reshape(128, 9 * 128),
        maskp.reshape(128, 3 * 32),
        maskm.reshape(128, 3 * 32),
    )


def build_nc(reps: int = 1, ablate: frozenset = frozenset()):
    nc = bacc.Bacc(None, target_bir_lowering=False)

    map_t = nc.dram_tensor("map_blk", [XS, 128, FREE], F16, kind="ExternalInput")
    aoh_t = nc.dram_tensor("a_oh", [XL, 128, NPR * 256], F8, kind="ExternalInput")
    boh_t = nc.dram_tensor("b_oh", [XL, 128, NPR * 256], F8, kind="ExternalInput")
    w_t = nc.dram_tensor("w27", [128, 27], F32, kind="ExternalInput")
    mask9_t = nc.dram_tensor("mask9", [128, 9 * 128], F16, kind="ExternalInput")
    maskp_t = nc.dram_tensor("maskp", [128, 96], F16, kind="ExternalInput")
    maskm_t = nc.dram_tensor("maskm", [128, 96], F16, kind="ExternalInput")
    out_t = nc.dram_tensor("out_blk", [XS, 128, FREE], F16, kind="ExternalOutput")

    with tile.TileContext(nc) as tc:
        with (
            tc.tile_pool(name="const", bufs=1) as cp,
            tc.tile_pool(name="ring", bufs=6) as ringp,
            tc.tile_pool(name="oh", bufs=3) as ohp,
            tc.tile_pool(name="mapio", bufs=3) as mapp,
            tc.tile_pool(name="hp", bufs=4, space="PSUM") as hpp,
            tc.tile_pool(name="cpm", bufs=3, space="PSUM") as cpp,
        ):
            # ---- constants ----
            mask9_sb = cp.tile([128, 9 * 128], F16)
            maskp_sb = cp.tile([128, 96], F16)
            maskm_sb = cp.tile([128, 96], F16)
            nc.sync.dma_start(out=mask9_sb[:], in_=mask9_t[:])
            nc.sync.dma_start(out=maskp_sb[:], in_=maskp_t[:])
            nc.sync.dma_start(out=maskm_sb[:], in_=maskm_t[:])

            # sigmoid(weights), host-replicated to all partitions; center -> 1
            w_bc = cp.tile([128, 27], F32)
            nc.sync.dma_start(out=w_bc[:], in_=w_t[:])
            nc.scalar.activation(out=w_bc[:], in_=w_bc[:], func=AF.Sigmoid)
            nc.vector.memset(w_bc[:, 13:14], 1.0)

            # banded stationaries: m0[fx] (128x128), mp[fx]/mm[fx] (128x32)
            m0 = [cp.tile([128, 128], F16, name=f"m0_{fx}") for fx in range(3)]
            mp = [cp.tile([128, 32], F16, name=f"mp_{fx}") for fx in range(3)]
            mm = [cp.tile([128, 32], F16, name=f"mm_{fx}") for fx in range(3)]
            tmp = cp.tile([128, 128], F16)
            for fx in range(3):
                for i, (fy, fz) in enumerate(
                    (fy, fz) for fy in range(3) for fz in range(3)
                ):
                    k = fy * 3 + fz
                    wcol = w_bc[:, fx * 9 + k : fx * 9 + k + 1]
                    dst = m0[fx][:] if i == 0 else tmp[:]
                    nc.vector.tensor_scalar(
                        out=dst,
                        in0=mask9_sb[:, k * 128 : (k + 1) * 128],
                        scalar1=wcol, scalar2=None, op0=ALU.mult,
                    )
                    if i > 0:
                        nc.vector.tensor_add(out=m0[fx][:], in0=m0[fx][:], in1=tmp[:])
                for fz in range(3):
                    wcol = w_bc[:, fx * 9 + 6 + fz : fx * 9 + 6 + fz + 1]
                    dstp = mp[fx][:] if fz == 0 else tmp[:, 0:32]
                    nc.vector.tensor_scalar(
                        out=dstp, in0=maskp_sb[:, fz * 32 : (fz + 1) * 32],
                        scalar1=wcol, scalar2=None, op0=ALU.mult,
                    )
                    if fz > 0:
                        nc.vector.tensor_add(
                            out=mp[fx][:], in0=mp[fx][:], in1=tmp[:, 0:32]
                        )
                    wcol = w_bc[:, fx * 9 + 0 + fz : fx * 9 + 0 + fz + 1]
                    dstm = mm[fx][:] if fz == 0 else tmp[:, 0:32]
                    nc.vector.tensor_scalar(
                        out=dstm, in0=maskm_sb[:, fz * 32 : (fz + 1) * 32],
                        scalar1=wcol, scalar2=None, op0=ALU.mult,
                    )
                    if fz > 0:
                        nc.vector.tensor_add(
                            out=mm[fx][:], in0=mm[fx][:], in1=tmp[:, 0:32]
                        )

            def one_pass():
                ring = [None] * XL
                oh_tiles = {}

                def fetch_oh(pp):
                    a = ohp.tile([128, NPR * 256], F8, tag="a_oh")
                    b = ohp.tile([128, NPR * 256], F8, tag="b_oh")
                    nc.scalar.dma_start(out=a[:], in_=aoh_t[pp])
                    nc.scalar.dma_start(out=b[:], in_=boh_t[pp])
                    oh_tiles[pp] = (a, b)

                fetch_oh(0)
                fetch_oh(1)
                for p in range(XL):
                    if p + 2 < XL:
                        fetch_oh(p + 2)
                    # prefetch current_map plane for out-plane q = p - 2
                    map_sb = None
                    if p >= 2:
                        map_sb = mapp.tile([128, FREE], F16, tag="map")
                        nc.sync.dma_start(out=map_sb[:], in_=map_t[p - 2])

                    # ---- histogram for hist-plane p (x_local = p-1) ----
                    a_oh, b_oh = oh_tiles.pop(p)
                    hp = [hpp.tile([128, 512], F32, name=f"hp_{p}_{j}", tag="hp")
                          for j in range(3)]
                    for pr in range(NPR):
                        bank, off = pr // 4, (pr % 4) * 128
                        nc.tensor.matmul(
                            out=hp[bank][:, off : off + 128],
                            lhsT=a_oh[:].rearrange(
                                "q (t two j) -> q t two j", two=2, j=128)[:, pr],
                            rhs=b_oh[:].rearrange(
                                "q (t two j) -> q t two j", two=2, j=128)[:, pr],
                            start=True, stop=True,
                            perf_mode=mybir.MatmulPerfMode.DoubleRow,
                        )
                    ring_t = ringp.tile([128, PLANE_F], F16, tag="ring")
                    ring[p] = ring_t
                    nc.gpsimd.memset(ring_t[:, 0:PAD], 0)
                    nc.gpsimd.memset(ring_t[:, PAD + FREE :], 0)
                    # evacuate psum -> fp16 plane: banks 0/1 on ACT, bank 2 DVE
                    for bank in range(2):
                        nc.scalar.activation(
                            out=ring_t[
                                :, PAD + bank * 8 * BW : PAD + (bank + 1) * 8 * BW
                            ].rearrange("q (s w) -> q s w", s=8),
                            in_=hp[bank][:]
                            .rearrange("q (s w) -> q s w", s=8)[:, :, 0:BW],
                            func=AF.Copy,
                        )
                    nc.vector.tensor_copy(
                        out=ring_t[:, PAD + 16 * BW : PAD + 21 * BW]
                        .rearrange("q (s w) -> q s w", s=5),
                        in_=hp[2][:]
                        .rearrange("q (s w) -> q s w", s=8)[:, 0:5, 0:BW],
                    )
                    nc.vector.tensor_copy(
                        out=ring_t[:, PAD + 21 * BW : PAD + FREE],
                        in_=hp[2][:, 5 * SLOT : 5 * SLOT + 21],
                    )

                    # ---- conv + map add for out-plane q = p - 2 ----
                    q = p - 2
                    if q < 0:
                        continue
                    cps = [cpp.tile([128, 512], F32, name=f"cp_{q}_{j}", tag="cp")
                           for j in range(3)]
                    for j, (off, w) in enumerate(CHUNKS):
                        for fx in range(3):
                            nc.tensor.matmul(
                                out=cps[j][:, 0:w],
                                lhsT=m0[fx][:],
                                rhs=ring[q + fx][:, PAD + off : PAD + off + w],
                                start=(fx == 0), stop=False,
                                skip_group_check=True,
                            )
                            nc.tensor.matmul(
                                out=cps[j][96:128, 0:w],
                                lhsT=mp[fx][:],
                                rhs=ring[q + fx][:, PAD + off + 21 : PAD + off + 21 + w],
                                start=False, stop=False,
                                tile_position=(0, 96),
                                skip_group_check=True,
                            )
                            nc.tensor.matmul(
                                out=cps[j][0:32, 0:w],
                                lhsT=mm[fx][:],
                                rhs=ring[q + fx][:, PAD + off - 21 : PAD + off - 21 + w],
                                start=False, stop=(fx == 2),
                                tile_position=(0, 0),
                                skip_group_check=True,
                            )
                    out_sb = mapp.tile([128, FREE], F16, tag="osb")
                    for j, (off, w) in enumerate(CHUNKS):
                        nc.vector.tensor_tensor(
                            out=out_sb[:, off : off + w],
                            in0=cps[j][:, 0:w],
                            in1=map_sb[:, off : off + w],
                            op=ALU.add,
                        )
                    nc.sync.dma_start(out=out_t[q], in_=out_sb[:])

            for _rep in range(reps):
                one_pass()
    nc.compile()
    return nc


# ---------------- host side ----------------

_NC_CACHE: dict[int, object] = {}
LAST_EXEC_NS = None


def _get_nc(reps: int = 1):
    if reps not in _NC_CACHE:
        _NC_CACHE[reps] = build_nc(reps)
    return _NC_CACHE[reps]


def _prep_inputs(current_map, point_cloud, weights):
    """Compute per-core in_maps + overflow list on the host."""
    mask9, maskp, maskm = _build_masks()
    w27 = np.ascontiguousarray(
        np.broadcast_to(weights.reshape(1, 27).astype(np.float32), (128, 27))
    )

    # blocked map: [x, (r,z), (g,c)]
    mb = np.ascontiguousarray(
        current_map.reshape(GX, GY // 4, 4, GZ, NC).transpose(0, 2, 3, 1, 4)
    ).reshape(GX, 128, FREE).astype(np.float16)

    xyz = point_cloud[:, :3]
    valid = np.all((xyz < MAX_B) & (xyz >= MIN_B), axis=1)
    inds = np.floor((xyz - MIN_B) / VOX).astype(np.int32)
    np.clip(inds, 0, np.array([GX - 1, GY - 1, GZ - 1], np.int32), out=inds)
    lab = np.clip(point_cloud[:, 3].astype(np.int32), 0, NC - 1)
    ix = inds[valid, 0]
    iy = inds[valid, 1]
    iz = inds[valid, 2]
    lab = lab[valid]

    a_all = (iy % 4) * 32 + iz
    b_all = ((iy % YB) // 4) * NC + lab
    bk_all = iy // YB

    in_maps = []
    overflow = []
    for c in range(N_CORES):
        x0 = XS * c
        sel = (ix >= x0 - 1) & (ix <= x0 + XS)
        cix, ciy, ciz, clab = ix[sel], iy[sel], iz[sel], lab[sel]
        t_arr = (cix - (x0 - 1)) * TPP + bk_all[sel]
        a_arr = a_all[sel]
        b_arr = b_all[sel]

        order = np.argsort(t_arr, kind="stable")
        ts, As, Bs = t_arr[order], a_arr[order], b_arr[order]
        counts = np.bincount(ts, minlength=T_TOT)
        starts = np.concatenate(([0], np.cumsum(counts)[:-1]))
        rank = np.arange(len(ts)) - starts[ts]
        ok = rank < 128
        pl, bk = ts[ok] // TPP, ts[ok] % TPP
        rk = rank[ok]
        pr, ki = bk // 2, bk % 2
        aoh = np.zeros(XL * 128 * NPR * 256, np.uint8)
        boh = np.zeros(XL * 128 * NPR * 256, np.uint8)
        base = ((pl * 128 + rk) * NPR + pr) * 256
        aoh[base + ki * 128 + As[ok]] = 0x38  # 1.0 in e4m3
        boh[base + ki * 192 + Bs[ok]] = 0x38
        aoh = aoh.view(ml_dtypes.float8_e4m3).reshape(XL, 128, NPR * 256)
        boh = boh.view(ml_dtypes.float8_e4m3).reshape(XL, 128, NPR * 256)
        if not ok.all():
            bad = order[~ok]
            for i_ in bad:
                overflow.append((c, cix[i_], ciy[i_], ciz[i_], clab[i_]))
        in_maps.append(
            {
                "map_blk": np.ascontiguousarray(mb[x0 : x0 + XS]),
                "a_oh": aoh,
                "b_oh": boh,
                "w27": w27,
                "mask9": mask9,
                "maskp": maskp,
                "maskm": maskm,
            }
        )
    return in_maps, overflow


def _apply_overflow(out, overflow, weights):
    if not overflow:
        return
    filt = 1.0 / (1.0 + np.exp(-weights.reshape(3, 3, 3).astype(np.float64)))
    filt = filt.astype(np.float32)
    filt[1, 1, 1] = 1.0
    for c, ix, iy, iz, lab in overflow:
        x0, x1 = XS * c, XS * (c + 1)
        for k0 in range(3):
            ox = ix + 1 - k0
            if ox < x0 or ox >= x1:
                continue
            for k1 in range(3):
                oy = iy + 1 - k1
                if oy < 0 or oy >= GY:
                    continue
                for k2 in range(3):
                    oz = iz + 1 - k2
                    if oz < 0 or oz >= GZ:
                        continue
                    out[ox, oy, oz, lab] += filt[k0, k1, k2]


def kernel(current_map, point_cloud, weights):
    global LAST_EXEC_NS
    current_map = np.asarray(current_map, np.float32)
    point_cloud = np.asarray(point_cloud, np.float32)
    weights = np.asarray(weights, np.float32)

    nc = _get_nc(1)
    in_maps, overflow = _prep_inputs(current_map, point_cloud, weights)
    res = run_bass_kernel_spmd(nc, in_maps, core_ids=list(range(N_CORES)))
    LAST_EXEC_NS = res.exec_time_ns

    out = np.empty((GX, GY, GZ, NC), np.float32)
    for c in range(N_CORES):
        blk = res.results[c]["out_blk"].astype(np.float32)  # [32, 128, 1344]
        out[XS * c : XS * (c + 1)] = (
            blk.reshape(XS, 4, 32, GY // 4, NC)
            .transpose(0, 3, 1, 2, 4)
            .reshape(XS, GY, GZ, NC)
        )
    _apply_overflow(out, overflow, weights)
    return out


# revision 23
# speedup vs baseline: 1.2526x; 1.2526x over previous
"""DiscreteBKI update kernel for Trainium2 (8 NeuronCores, Bass/Tile).

Pipeline (per core, x-slab of 32 planes + 1-plane halo each side):
  1. host: bucket valid points by (x-plane, y-block-of-12), precompute
     per-point (a, b) sub-indices for the on-device scatter.
  2. device: histogram scatter via one-hot matmuls accumulating in PSUM
     (exact: one-hot fp16 products accumulated in fp32).
  3. device: 3x3x3 conv as 9 banded matmuls per output plane over a
     (y%4, z) x (y//4, class) blocked layout, fused with current_map add.
  4. host: un-block the 8 output slabs (f16) into the full map (f32).

Layout: y = 4g + r;  SBUF partition p = r*32 + z;  free col f = g*21 + c.
"""

import os
import sys

import numpy as np
import ml_dtypes

for _p in (
    "/opt/trn_rl_repo",
    "/root/.axon_site/_ro/trn_rl_repo",
    "/root/.axon_site",
    "/root/.axon_site/_ro/pypackages",
):
    if os.path.isdir(_p) and _p not in sys.path:
        sys.path.append(_p)

import concourse.bacc as bacc  # noqa: E402
import concourse.mybir as mybir  # noqa: E402
import concourse.tile as tile  # noqa: E402
from concourse.bass_utils import run_bass_kernel_spmd  # noqa: E402

F16 = mybir.dt.float16
F32 = mybir.dt.float32
F8 = mybir.dt.float8e4
AF = mybir.ActivationFunctionType
ALU = mybir.AluOpType

# ---- problem geometry (hardcoded; must match the reference) ----
GX, GY, GZ, NC = 256, 256, 32, 21
MIN_B = np.array([-25.6, -25.6, -2.0], np.float32)
MAX_B = np.array([25.6, 25.6, 1.2], np.float32)
VOX = (MAX_B - MIN_B) / np.array([GX, GY, GZ], np.float32)
N_CORES = 8
XS = GX // N_CORES            # 32 x-planes owned per core
XL = XS + 2                   # 34 hist planes (with +-1 halo)
YB = 12                       # y-block per scatter bucket
NBK = 22                      # buckets per plane (21 full + 1 of width 4)
BW = 63                       # b-range per bucket (3 * 21)
SLOT = 64                     # psum cols reserved per bucket
FREE = (GY // 4) * NC         # 1344
PAD = NC                      # 21 zero cols each side of a plane tile
PLANE_F = FREE + 2 * PAD      # 1386
TPP = NBK                     # point tiles per plane (1 tile per bucket)
NPR = NBK // 2                # DoubleRow bucket-pairs per plane (11)
T_TOT = XL * TPP              # 748 point tiles per core
CHUNKS = ((0, 512), (512, 512), (1024, FREE - 1024))


def _build_masks():
    """Constant selection masks for assembling banded conv stationaries."""
    p = np.arange(128)
    r_in, z_in = p >> 5, p & 31
    m = np.arange(128)
    r_out, z_out = m >> 5, m & 31
    mask9 = np.zeros((128, 9, 128), np.float16)
    for fy in range(3):
        for fz in range(3):
            mask9[:, fy * 3 + fz, :] = (
                (r_in[:, None] - r_out[None, :] == fy - 1)
                & (z_in[:, None] - z_out[None, :] == fz - 1)
            )
    zo = np.arange(32)
    maskp = np.zeros((128, 3, 32), np.float16)
    maskm = np.zeros((128, 3, 32), np.float16)
    for fz in range(3):
        maskp[:, fz, :] = (p[:, None] < 32) & (p[:, None] - zo[None, :] == fz - 1)
        maskm[:, fz, :] = (p[:, None] >= 96) & (
            (p[:, None] - 96) - zo[None, :] == fz - 1
        )
    return (
        mask9.reshape(128, 9 * 128),
        maskp.reshape(128, 3 * 32),
        maskm.reshape(128, 3 * 32),
    )


def build_nc(reps: int = 1, ablate: frozenset = frozenset()):
    nc = bacc.Bacc(None, target_bir_lowering=False)

    map_t = nc.dram_tensor("map_blk", [XS, 128, FREE], F16, kind="ExternalInput")
    aoh_t = nc.dram_tensor("a_oh", [XL, 128, NPR * 256], F8, kind="ExternalInput")
    boh_t = nc.dram_tensor("b_oh", [XL, 128, NPR * 256], F8, kind="ExternalInput")
    w_t = nc.dram_tensor("w27", [128, 27], F32, kind="ExternalInput")
    mask9_t = nc.dram_tensor("mask9", [128, 9 * 128], F16, kind="ExternalInput")
    maskp_t = nc.dram_tensor("maskp", [128, 96], F16, kind="ExternalInput")
    maskm_t = nc.dram_tensor("maskm", [128, 96], F16, kind="ExternalInput")
    out_t = nc.dram_tensor("out_blk", [XS, 128, FREE], F16, kind="ExternalOutput")

    with tile.TileContext(nc) as tc:
        with (
            tc.tile_pool(name="const", bufs=1) as cp,
            tc.tile_pool(name="ring", bufs=6) as ringp,
            tc.tile_pool(name="oh", bufs=3) as ohp,
            tc.tile_pool(name="mapio", bufs=3) as mapp,
            tc.tile_pool(name="hp", bufs=4, space="PSUM") as hpp,
            tc.tile_pool(name="cpm", bufs=3, space="PSUM") as cpp,
        ):
            # ---- constants ----
            mask9_sb = cp.tile([128, 9 * 128], F16)
            maskp_sb = cp.tile([128, 96], F16)
            maskm_sb = cp.tile([128, 96], F16)
            nc.sync.dma_start(out=mask9_sb[:], in_=mask9_t[:])
            nc.sync.dma_start(out=maskp_sb[:], in_=maskp_t[:])
            nc.sync.dma_start(out=maskm_sb[:], in_=maskm_t[:])

            # sigmoid(weights), host-replicated to all partitions; center -> 1
            w_bc = cp.tile([128, 27], F32)
            nc.sync.dma_start(out=w_bc[:], in_=w_t[:])
            nc.scalar.activation(out=w_bc[:], in_=w_bc[:], func=AF.Sigmoid)
            nc.vector.memset(w_bc[:, 13:14], 1.0)

            # banded stationaries: m0[fx] (128x128), mp[fx]/mm[fx] (128x32)
            m0 = [cp.tile([128, 128], F16, name=f"m0_{fx}") for fx in range(3)]
            mp = [cp.tile([128, 32], F16, name=f"mp_{fx}") for fx in range(3)]
            mm = [cp.tile([128, 32], F16, name=f"mm_{fx}") for fx in range(3)]
            tmp = cp.tile([128, 128], F16)
            for fx in range(3):
                for i, (fy, fz) in enumerate(
                    (fy, fz) for fy in range(3) for fz in range(3)
                ):
                    k = fy * 3 + fz
                    wcol = w_bc[:, fx * 9 + k : fx * 9 + k + 1]
                    dst = m0[fx][:] if i == 0 else tmp[:]
                    nc.vector.tensor_scalar(
                        out=dst,
                        in0=mask9_sb[:, k * 128 : (k + 1) * 128],
                        scalar1=wcol, scalar2=None, op0=ALU.mult,
                    )
                    if i > 0:
                        nc.vector.tensor_add(out=m0[fx][:], in0=m0[fx][:], in1=tmp[:])
                for fz in range(3):
                    wcol = w_bc[:, fx * 9 + 6 + fz : fx * 9 + 6 + fz + 1]
                    dstp = mp[fx][:] if fz == 0 else tmp[:, 0:32]
                    nc.vector.tensor_scalar(
                        out=dstp, in0=maskp_sb[:, fz * 32 : (fz + 1) * 32],
                        scalar1=wcol, scalar2=None, op0=ALU.mult,
                    )
                    if fz > 0:
                        nc.vector.tensor_add(
                            out=mp[fx][:], in0=mp[fx][:], in1=tmp[:, 0:32]
                        )
                    wcol = w_bc[:, fx * 9 + 0 + fz : fx * 9 + 0 + fz + 1]
                    dstm = mm[fx][:] if fz == 0 else tmp[:, 0:32]
                    nc.vector.tensor_scalar(
                        out=dstm, in0=maskm_sb[:, fz * 32 : (fz + 1) * 32],
                        scalar1=wcol, scalar2=None, op0=ALU.mult,
                    )
                    if fz > 0:
                        nc.vector.tensor_add(
                            out=mm[fx][:], in0=mm[fx][:], in1=tmp[:, 0:32]
                        )

            def one_pass():
                ring = [None] * XL
                oh_tiles = {}

                def fetch_oh(pp):
                    a = ohp.tile([128, NPR * 256], F8, tag="a_oh")
                    b = ohp.tile([128, NPR * 256], F8, tag="b_oh")
                    nc.scalar.dma_start(out=a[:], in_=aoh_t[pp])
                    nc.scalar.dma_start(out=b[:], in_=boh_t[pp])
                    oh_tiles[pp] = (a, b)

                fetch_oh(0)
                fetch_oh(1)
                for p in range(XL):
                    if p + 2 < XL:
                        fetch_oh(p + 2)
                    # prefetch current_map plane for out-plane q = p - 2
                    map_sb = None
                    if p >= 2:
                        map_sb = mapp.tile([128, FREE], F16, tag="map")
                        nc.sync.dma_start(out=map_sb[:], in_=map_t[p - 2])

                    # ---- histogram for hist-plane p (x_local = p-1) ----
                    a_oh, b_oh = oh_tiles.pop(p)
                    hp = [hpp.tile([128, 512], F32, name=f"hp_{p}_{j}", tag="hp")
                          for j in range(3)]
                    for pr in range(NPR):
                        bank, off = pr // 4, (pr % 4) * 128
                        nc.tensor.matmul(
                            out=hp[bank][:, off : off + 128],
                            lhsT=a_oh[:].rearrange(
                                "q (t two j) -> q t two j", two=2, j=128)[:, pr],
                            rhs=b_oh[:].rearrange(
                                "q (t two j) -> q t two j", two=2, j=128)[:, pr],
                            start=True, stop=True,
                            perf_mode=mybir.MatmulPerfMode.DoubleRow,
                        )
                    ring_t = ringp.tile([128, PLANE_F], F16, tag="ring")
                    ring[p] = ring_t
                    nc.gpsimd.memset(ring_t[:, 0:PAD], 0)
                    nc.gpsimd.memset(ring_t[:, PAD + FREE :], 0)
                    # evacuate psum -> fp16 plane: banks 0/1 on ACT, bank 2 DVE
                    for bank in range(2):
                        nc.scalar.activation(
                            out=ring_t[
                                :, PAD + bank * 8 * BW : PAD + (bank + 1) * 8 * BW
                            ].rearrange("q (s w) -> q s w", s=8),
                            in_=hp[bank][:]
                            .rearrange("q (s w) -> q s w", s=8)[:, :, 0:BW],
                            func=AF.Copy,
                        )
                    nc.vector.tensor_copy(
                        out=ring_t[:, PAD + 16 * BW : PAD + 21 * BW]
                        .rearrange("q (s w) -> q s w", s=5),
                        in_=hp[2][:]
                        .rearrange("q (s w) -> q s w", s=8)[:, 0:5, 0:BW],
                    )
                    nc.vector.tensor_copy(
                        out=ring_t[:, PAD + 21 * BW : PAD + FREE],
                        in_=hp[2][:, 5 * SLOT : 5 * SLOT + 21],
                    )

                    # ---- conv + map add for out-plane q = p - 2 ----
                    q = p - 2
                    if q < 0:
                        continue
                    cps = [cpp.tile([128, 512], F32, name=f"cp_{q}_{j}", tag="cp")
                           for j in range(3)]
                    for j, (off, w) in enumerate(CHUNKS):
                        for fx in range(3):
                            nc.tensor.matmul(
                                out=cps[j][:, 0:w],
                                lhsT=m0[fx][:],
                                rhs=ring[q + fx][:, PAD + off : PAD + off + w],
                                start=(fx == 0), stop=False,
                                skip_group_check=True,
                            )
                        for fx in range(3):
                            nc.tensor.matmul(
                                out=cps[j][96:128, 0:w],
                                lhsT=mp[fx][:],
                                rhs=ring[q + fx][:, PAD + off + 21 : PAD + off + 21 + w],
                                start=False, stop=False,
                                tile_position=(0, 96),
                                skip_group_check=True,
                            )
                            nc.tensor.matmul(
                                out=cps[j][0:32, 0:w],
                                lhsT=mm[fx][:],
                                rhs=ring[q + fx][:, PAD + off - 21 : PAD + off - 21 + w],
                                start=False, stop=(fx == 2),
                                tile_position=(0, 0),
                                skip_group_check=True,
                            )
                    out_sb = mapp.tile([128, FREE], F16, tag="osb")
                    for j, (off, w) in enumerate(CHUNKS):
                        nc.vector.tensor_tensor(
                            out=out_sb[:, off : off + w],
                            in0=cps[j][:, 0:w],
                            in1=map_sb[:, off : off + w],
                            op=ALU.add,
                        )
                    nc.sync.dma_start(out=out_t[q], in_=out_sb[:])

            for _rep in range(reps):
                one_pass()
    nc.compile()
    return nc


# ---------------- host side ----------------

_NC_CACHE: dict[int, object] = {}
LAST_EXEC_NS = None


def _get_nc(reps: int = 1):
    if reps not in _NC_CACHE:
        _NC_CACHE[reps] = build_nc(reps)
    return _NC_CACHE[reps]


def _prep_inputs(current_map, point_cloud, weights):
    """Compute per-core in_maps + overflow list on the host."""
    mask9, maskp, maskm = _build_masks()
    w27 = np.ascontiguousarray(
        np.broadcast_to(weights.reshape(1, 27).astype(np.float32), (128, 27))
    )

    # blocked map: [x, (r,z), (g,c)]
    mb = np.ascontiguousarray(
        current_map.reshape(GX, GY // 4, 4, GZ, NC).transpose(0, 2, 3, 1, 4)
    ).reshape(GX, 128, FREE).astype(np.float16)

    xyz = point_cloud[:, :3]
    valid = np.all((xyz < MAX_B) & (xyz >= MIN_B), axis=1)
    inds = np.floor((xyz - MIN_B) / VOX).astype(np.int32)
    np.clip(inds, 0, np.array([GX - 1, GY - 1, GZ - 1], np.int32), out=inds)
    lab = np.clip(point_cloud[:, 3].astype(np.int32), 0, NC - 1)
    ix = inds[valid, 0]
    iy = inds[valid, 1]
    iz = inds[valid, 2]
    lab = lab[valid]

    a_all = (iy % 4) * 32 + iz
    b_all = ((iy % YB) // 4) * NC + lab
    bk_all = iy // YB

    in_maps = []
    overflow = []
    for c in range(N_CORES):
        x0 = XS * c
        sel = (ix >= x0 - 1) & (ix <= x0 + XS)
        cix, ciy, ciz, clab = ix[sel], iy[sel], iz[sel], lab[sel]
        t_arr = (cix - (x0 - 1)) * TPP + bk_all[sel]
        a_arr = a_all[sel]
        b_arr = b_all[sel]

        order = np.argsort(t_arr, kind="stable")
        ts, As, Bs = t_arr[order], a_arr[order], b_arr[order]
        counts = np.bincount(ts, minlength=T_TOT)
        starts = np.concatenate(([0], np.cumsum(counts)[:-1]))
        rank = np.arange(len(ts)) - starts[ts]
        ok = rank < 128
        pl, bk = ts[ok] // TPP, ts[ok] % TPP
        rk = rank[ok]
        pr, ki = bk // 2, bk % 2
        aoh = np.zeros(XL * 128 * NPR * 256, np.uint8)
        boh = np.zeros(XL * 128 * NPR * 256, np.uint8)
        base = ((pl * 128 + rk) * NPR + pr) * 256
        aoh[base + ki * 128 + As[ok]] = 0x38  # 1.0 in e4m3
        boh[base + ki * 192 + Bs[ok]] = 0x38
        aoh = aoh.view(ml_dtypes.float8_e4m3).reshape(XL, 128, NPR * 256)
        boh = boh.view(ml_dtypes.float8_e4m3).reshape(XL, 128, NPR * 256)
        if not ok.all():
            bad = order[~ok]
            for i_ in bad:
                overflow.append((c, cix[i_], ciy[i_], ciz[i_], clab[i_]))
        in_maps.append(
            {
                "map_blk": np.ascontiguousarray(mb[x0 : x0 + XS]),
                "a_oh": aoh,
                "b_oh": boh,
                "w27": w27,
                "mask9": mask9,
                "maskp": maskp,
                "maskm": maskm,
            }
        )
    return in_maps, overflow


def _apply_overflow(out, overflow, weights):
    if not overflow:
        return
    filt = 1.0 / (1.0 + np.exp(-weights.reshape(3, 3, 3).astype(np.float64)))
    filt = filt.astype(np.float32)
    filt[1, 1, 1] = 1.0
    for c, ix, iy, iz, lab in overflow:
        x0, x1 = XS * c, XS * (c + 1)
        for k0 in range(3):
            ox = ix + 1 - k0
            if ox < x0 or ox >= x1:
                continue
            for k1 in range(3):
                oy = iy + 1 - k1
                if oy < 0 or oy >= GY:
                    continue
                for k2 in range(3):
                    oz = iz + 1 - k2
                    if oz < 0 or oz >= GZ:
                        continue
                    out[ox, oy, oz, lab] += filt[k0, k1, k2]


def kernel(current_map, point_cloud, weights):
    global LAST_EXEC_NS
    current_map = np.asarray(current_map, np.float32)
    point_cloud = np.asarray(point_cloud, np.float32)
    weights = np.asarray(weights, np.float32)

    nc = _get_nc(1)
    in_maps, overflow = _prep_inputs(current_map, point_cloud, weights)
    res = run_bass_kernel_spmd(nc, in_maps, core_ids=list(range(N_CORES)))
    LAST_EXEC_NS = res.exec_time_ns

    out = np.empty((GX, GY, GZ, NC), np.float32)
    for c in range(N_CORES):
        blk = res.results[c]["out_blk"].astype(np.float32)  # [32, 128, 1344]
        out[XS * c : XS * (c + 1)] = (
            blk.reshape(XS, 4, 32, GY // 4, NC)
            .transpose(0, 3, 1, 2, 4)
            .reshape(XS, GY, GZ, NC)
        )
    _apply_overflow(out, overflow, weights)
    return out


# revision 24
# speedup vs baseline: 1.2661x; 1.0107x over previous
"""DiscreteBKI update kernel for Trainium2 (8 NeuronCores, Bass/Tile).

Pipeline (per core, x-slab of 32 planes + 1-plane halo each side):
  1. host: bucket valid points by (x-plane, y-block-of-12), precompute
     per-point (a, b) sub-indices for the on-device scatter.
  2. device: histogram scatter via one-hot matmuls accumulating in PSUM
     (exact: one-hot fp16 products accumulated in fp32).
  3. device: 3x3x3 conv as 9 banded matmuls per output plane over a
     (y%4, z) x (y//4, class) blocked layout, fused with current_map add.
  4. host: un-block the 8 output slabs (f16) into the full map (f32).

Layout: y = 4g + r;  SBUF partition p = r*32 + z;  free col f = g*21 + c.
"""

import os
import sys

import numpy as np
import ml_dtypes

for _p in (
    "/opt/trn_rl_repo",
    "/root/.axon_site/_ro/trn_rl_repo",
    "/root/.axon_site",
    "/root/.axon_site/_ro/pypackages",
):
    if os.path.isdir(_p) and _p not in sys.path:
        sys.path.append(_p)

import concourse.bacc as bacc  # noqa: E402
import concourse.mybir as mybir  # noqa: E402
import concourse.tile as tile  # noqa: E402
from concourse.bass_utils import run_bass_kernel_spmd  # noqa: E402

F16 = mybir.dt.float16
F32 = mybir.dt.float32
F8 = mybir.dt.float8e4
AF = mybir.ActivationFunctionType
ALU = mybir.AluOpType

# ---- problem geometry (hardcoded; must match the reference) ----
GX, GY, GZ, NC = 256, 256, 32, 21
MIN_B = np.array([-25.6, -25.6, -2.0], np.float32)
MAX_B = np.array([25.6, 25.6, 1.2], np.float32)
VOX = (MAX_B - MIN_B) / np.array([GX, GY, GZ], np.float32)
N_CORES = 8
XS = GX // N_CORES            # 32 x-planes owned per core
XL = XS + 2                   # 34 hist planes (with +-1 halo)
YB = 12                       # y-block per scatter bucket
NBK = 22                      # buckets per plane (21 full + 1 of width 4)
BW = 63                       # b-range per bucket (3 * 21)
SLOT = 64                     # psum cols reserved per bucket
FREE = (GY // 4) * NC         # 1344
PAD = NC                      # 21 zero cols each side of a plane tile
PLANE_F = FREE + 2 * PAD      # 1386
TPP = NBK                     # point tiles per plane (1 tile per bucket)
NPR = NBK // 2                # DoubleRow bucket-pairs per plane (11)
T_TOT = XL * TPP              # 748 point tiles per core
CHUNKS = ((0, 512), (512, 512), (1024, FREE - 1024))


def _build_masks():
    """Constant selection masks for assembling banded conv stationaries."""
    p = np.arange(128)
    r_in, z_in = p >> 5, p & 31
    m = np.arange(128)
    r_out, z_out = m >> 5, m & 31
    mask9 = np.zeros((128, 9, 128), np.float16)
    for fy in range(3):
        for fz in range(3):
            mask9[:, fy * 3 + fz, :] = (
                (r_in[:, None] - r_out[None, :] == fy - 1)
                & (z_in[:, None] - z_out[None, :] == fz - 1)
            )
    zo = np.arange(32)
    maskp = np.zeros((128, 3, 32), np.float16)
    maskm = np.zeros((128, 3, 32), np.float16)
    for fz in range(3):
        maskp[:, fz, :] = (p[:, None] < 32) & (p[:, None] - zo[None, :] == fz - 1)
        maskm[:, fz, :] = (p[:, None] >= 96) & (
            (p[:, None] - 96) - zo[None, :] == fz - 1
        )
    return (
        mask9.reshape(128, 9 * 128),
        maskp.reshape(128, 3 * 32),
        maskm.reshape(128, 3 * 32),
    )


def build_nc(reps: int = 1, ablate: frozenset = frozenset()):
    nc = bacc.Bacc(None, target_bir_lowering=False)

    map_t = nc.dram_tensor("map_blk", [XS, 128, FREE], F16, kind="ExternalInput")
    aoh_t = nc.dram_tensor("a_oh", [XL, 128, NPR * 256], F8, kind="ExternalInput")
    boh_t = nc.dram_tensor("b_oh", [XL, 128, NPR * 256], F8, kind="ExternalInput")
    w_t = nc.dram_tensor("w27", [128, 27], F32, kind="ExternalInput")
    mask9_t = nc.dram_tensor("mask9", [128, 9 * 128], F16, kind="ExternalInput")
    maskp_t = nc.dram_tensor("maskp", [128, 96], F16, kind="ExternalInput")
    maskm_t = nc.dram_tensor("maskm", [128, 96], F16, kind="ExternalInput")
    out_t = nc.dram_tensor("out_blk", [XS, 128, FREE], F16, kind="ExternalOutput")

    with tile.TileContext(nc) as tc:
        with (
            tc.tile_pool(name="const", bufs=1) as cp,
            tc.tile_pool(name="ring", bufs=6) as ringp,
            tc.tile_pool(name="oh", bufs=3) as ohp,
            tc.tile_pool(name="mapio", bufs=3) as mapp,
            tc.tile_pool(name="hp", bufs=4, space="PSUM") as hpp,
            tc.tile_pool(name="cpm", bufs=3, space="PSUM") as cpp,
        ):
            # ---- constants ----
            mask9_sb = cp.tile([128, 9 * 128], F16)
            maskp_sb = cp.tile([128, 96], F16)
            maskm_sb = cp.tile([128, 96], F16)
            nc.sync.dma_start(out=mask9_sb[:], in_=mask9_t[:])
            nc.sync.dma_start(out=maskp_sb[:], in_=maskp_t[:])
            nc.sync.dma_start(out=maskm_sb[:], in_=maskm_t[:])

            # prologue one-hot fetches first: the scalar queue issues these
            # before the sigmoid's ACT table load so plane 0 can start early
            oh_tiles = {}

            def fetch_oh(pp):
                a = ohp.tile([128, NPR * 256], F8, tag="a_oh")
                b = ohp.tile([128, NPR * 256], F8, tag="b_oh")
                nc.scalar.dma_start(out=a[:], in_=aoh_t[pp])
                nc.scalar.dma_start(out=b[:], in_=boh_t[pp])
                oh_tiles[pp] = (a, b)

            fetch_oh(0)
            fetch_oh(1)

            # sigmoid(weights), host-replicated to all partitions; center -> 1
            w_bc = cp.tile([128, 27], F32)
            nc.sync.dma_start(out=w_bc[:], in_=w_t[:])
            nc.scalar.activation(out=w_bc[:], in_=w_bc[:], func=AF.Sigmoid)
            nc.vector.memset(w_bc[:, 13:14], 1.0)

            # banded stationaries: m0[fx] (128x128), mp[fx]/mm[fx] (128x32)
            m0 = [cp.tile([128, 128], F16, name=f"m0_{fx}") for fx in range(3)]
            mp = [cp.tile([128, 32], F16, name=f"mp_{fx}") for fx in range(3)]
            mm = [cp.tile([128, 32], F16, name=f"mm_{fx}") for fx in range(3)]
            tmp = cp.tile([128, 128], F16)
            for fx in range(3):
                for i, (fy, fz) in enumerate(
                    (fy, fz) for fy in range(3) for fz in range(3)
                ):
                    k = fy * 3 + fz
                    wcol = w_bc[:, fx * 9 + k : fx * 9 + k + 1]
                    dst = m0[fx][:] if i == 0 else tmp[:]
                    nc.vector.tensor_scalar(
                        out=dst,
                        in0=mask9_sb[:, k * 128 : (k + 1) * 128],
                        scalar1=wcol, scalar2=None, op0=ALU.mult,
                    )
                    if i > 0:
                        nc.vector.tensor_add(out=m0[fx][:], in0=m0[fx][:], in1=tmp[:])
                for fz in range(3):
                    wcol = w_bc[:, fx * 9 + 6 + fz : fx * 9 + 6 + fz + 1]
                    dstp = mp[fx][:] if fz == 0 else tmp[:, 0:32]
                    nc.vector.tensor_scalar(
                        out=dstp, in0=maskp_sb[:, fz * 32 : (fz + 1) * 32],
                        scalar1=wcol, scalar2=None, op0=ALU.mult,
                    )
                    if fz > 0:
                        nc.vector.tensor_add(
                            out=mp[fx][:], in0=mp[fx][:], in1=tmp[:, 0:32]
                        )
                    wcol = w_bc[:, fx * 9 + 0 + fz : fx * 9 + 0 + fz + 1]
                    dstm = mm[fx][:] if fz == 0 else tmp[:, 0:32]
                    nc.vector.tensor_scalar(
                        out=dstm, in0=maskm_sb[:, fz * 32 : (fz + 1) * 32],
                        scalar1=wcol, scalar2=None, op0=ALU.mult,
                    )
                    if fz > 0:
                        nc.vector.tensor_add(
                            out=mm[fx][:], in0=mm[fx][:], in1=tmp[:, 0:32]
                        )

            def one_pass():
                ring = [None] * XL
                for p in range(XL):
                    if p + 2 < XL:
                        fetch_oh(p + 2)
                    # prefetch current_map plane for out-plane q = p - 2
                    map_sb = None
                    if p >= 2:
                        map_sb = mapp.tile([128, FREE], F16, tag="map")
                        nc.sync.dma_start(out=map_sb[:], in_=map_t[p - 2])

                    # ---- histogram for hist-plane p (x_local = p-1) ----
                    a_oh, b_oh = oh_tiles.pop(p)
                    hp = [hpp.tile([128, 512], F32, name=f"hp_{p}_{j}", tag="hp")
                          for j in range(3)]
                    for pr in range(NPR):
                        bank, off = pr // 4, (pr % 4) * 128
                        nc.tensor.matmul(
                            out=hp[bank][:, off : off + 128],
                            lhsT=a_oh[:].rearrange(
                                "q (t two j) -> q t two j", two=2, j=128)[:, pr],
                            rhs=b_oh[:].rearrange(
                                "q (t two j) -> q t two j", two=2, j=128)[:, pr],
                            start=True, stop=True,
                            perf_mode=mybir.MatmulPerfMode.DoubleRow,
                        )
                    ring_t = ringp.tile([128, PLANE_F], F16, tag="ring")
                    ring[p] = ring_t
                    nc.gpsimd.memset(ring_t[:, 0:PAD], 0)
                    nc.gpsimd.memset(ring_t[:, PAD + FREE :], 0)
                    # evacuate psum -> fp16 plane: banks 0/1 on ACT, bank 2 DVE
                    for bank in range(2):
                        nc.scalar.activation(
                            out=ring_t[
                                :, PAD + bank * 8 * BW : PAD + (bank + 1) * 8 * BW
                            ].rearrange("q (s w) -> q s w", s=8),
                            in_=hp[bank][:]
                            .rearrange("q (s w) -> q s w", s=8)[:, :, 0:BW],
                            func=AF.Copy,
                        )
                    nc.vector.tensor_copy(
                        out=ring_t[:, PAD + 16 * BW : PAD + 21 * BW]
                        .rearrange("q (s w) -> q s w", s=5),
                        in_=hp[2][:]
                        .rearrange("q (s w) -> q s w", s=8)[:, 0:5, 0:BW],
                    )
                    nc.vector.tensor_copy(
                        out=ring_t[:, PAD + 21 * BW : PAD + FREE],
                        in_=hp[2][:, 5 * SLOT : 5 * SLOT + 21],
                    )

                    # ---- conv + map add for out-plane q = p - 2 ----
                    q = p - 2
                    if q < 0:
                        continue
                    cps = [cpp.tile([128, 512], F32, name=f"cp_{q}_{j}", tag="cp")
                           for j in range(3)]
                    for j, (off, w) in enumerate(CHUNKS):
                        for fx in range(3):
                            nc.tensor.matmul(
                                out=cps[j][:, 0:w],
                                lhsT=m0[fx][:],
                                rhs=ring[q + fx][:, PAD + off : PAD + off + w],
                                start=(fx == 0), stop=False,
                                skip_group_check=True,
                            )
                        for fx in range(3):
                            nc.tensor.matmul(
                                out=cps[j][96:128, 0:w],
                                lhsT=mp[fx][:],
                                rhs=ring[q + fx][:, PAD + off + 21 : PAD + off + 21 + w],
                                start=False, stop=False,
                                tile_position=(0, 96),
                                skip_group_check=True,
                            )
                            nc.tensor.matmul(
                                out=cps[j][0:32, 0:w],
                                lhsT=mm[fx][:],
                                rhs=ring[q + fx][:, PAD + off - 21 : PAD + off - 21 + w],
                                start=False, stop=(fx == 2),
                                tile_position=(0, 0),
                                skip_group_check=True,
                            )
                    out_sb = mapp.tile([128, FREE], F16, tag="osb")
                    for j, (off, w) in enumerate(CHUNKS):
                        nc.vector.tensor_tensor(
                            out=out_sb[:, off : off + w],
                            in0=cps[j][:, 0:w],
                            in1=map_sb[:, off : off + w],
                            op=ALU.add,
                        )
                    nc.sync.dma_start(out=out_t[q], in_=out_sb[:])

            for _rep in range(reps):
                one_pass()
    nc.compile()
    return nc


# ---------------- host side ----------------

_NC_CACHE: dict[int, object] = {}
LAST_EXEC_NS = None


def _get_nc(reps: int = 1):
    if reps not in _NC_CACHE:
        _NC_CACHE[reps] = build_nc(reps)
    return _NC_CACHE[reps]


def _prep_inputs(current_map, point_cloud, weights):
    """Compute per-core in_maps + overflow list on the host."""
    mask9, maskp, maskm = _build_masks()
    w27 = np.ascontiguousarray(
        np.broadcast_to(weights.reshape(1, 27).astype(np.float32), (128, 27))
    )

    # blocked map: [x, (r,z), (g,c)]
    mb = np.ascontiguousarray(
        current_map.reshape(GX, GY // 4, 4, GZ, NC).transpose(0, 2, 3, 1, 4)
    ).reshape(GX, 128, FREE).astype(np.float16)

    xyz = point_cloud[:, :3]
    valid = np.all((xyz < MAX_B) & (xyz >= MIN_B), axis=1)
    inds = np.floor((xyz - MIN_B) / VOX).astype(np.int32)
    np.clip(inds, 0, np.array([GX - 1, GY - 1, GZ - 1], np.int32), out=inds)
    lab = np.clip(point_cloud[:, 3].astype(np.int32), 0, NC - 1)
    ix = inds[valid, 0]
    iy = inds[valid, 1]
    iz = inds[valid, 2]
    lab = lab[valid]

    a_all = (iy % 4) * 32 + iz
    b_all = ((iy % YB) // 4) * NC + lab
    bk_all = iy // YB

    in_maps = []
    overflow = []
    for c in range(N_CORES):
        x0 = XS * c
        sel = (ix >= x0 - 1) & (ix <= x0 + XS)
        cix, ciy, ciz, clab = ix[sel], iy[sel], iz[sel], lab[sel]
        t_arr = (cix - (x0 - 1)) * TPP + bk_all[sel]
        a_arr = a_all[sel]
        b_arr = b_all[sel]

        order = np.argsort(t_arr, kind="stable")
        ts, As, Bs = t_arr[order], a_arr[order], b_arr[order]
        counts = np.bincount(ts, minlength=T_TOT)
        starts = np.concatenate(([0], np.cumsum(counts)[:-1]))
        rank = np.arange(len(ts)) - starts[ts]
        ok = rank < 128
        pl, bk = ts[ok] // TPP, ts[ok] % TPP
        rk = rank[ok]
        pr, ki = bk // 2, bk % 2
        aoh = np.zeros(XL * 128 * NPR * 256, np.uint8)
        boh = np.zeros(XL * 128 * NPR * 256, np.uint8)
        base = ((pl * 128 + rk) * NPR + pr) * 256
        aoh[base + ki * 128 + As[ok]] = 0x38  # 1.0 in e4m3
        boh[base + ki * 192 + Bs[ok]] = 0x38
        aoh = aoh.view(ml_dtypes.float8_e4m3).reshape(XL, 128, NPR * 256)
        boh = boh.view(ml_dtypes.float8_e4m3).reshape(XL, 128, NPR * 256)
        if not ok.all():
            bad = order[~ok]
            for i_ in bad:
                overflow.append((c, cix[i_], ciy[i_], ciz[i_], clab[i_]))
        in_maps.append(
            {
                "map_blk": np.ascontiguousarray(mb[x0 : x0 + XS]),
                "a_oh": aoh,
                "b_oh": boh,
                "w27": w27,
                "mask9": mask9,
                "maskp": maskp,
                "maskm": maskm,
            }
        )
    return in_maps, overflow


def _apply_overflow(out, overflow, weights):
    if not overflow:
        return
    filt = 1.0 / (1.0 + np.exp(-weights.reshape(3, 3, 3).astype(np.float64)))
    filt = filt.astype(np.float32)
    filt[1, 1, 1] = 1.0
    for c, ix, iy, iz, lab in overflow:
        x0, x1 = XS * c, XS * (c + 1)
        for k0 in range(3):
            ox = ix + 1 - k0
            if ox < x0 or ox >= x1:
                continue
            for k1 in range(3):
                oy = iy + 1 - k1
                if oy < 0 or oy >= GY:
                    continue
                for k2 in range(3):
                    oz = iz + 1 - k2
                    if oz < 0 or oz >= GZ:
                        continue
                    out[ox, oy, oz, lab] += filt[k0, k1, k2]


def kernel(current_map, point_cloud, weights):
    global LAST_EXEC_NS
    current_map = np.asarray(current_map, np.float32)
    point_cloud = np.asarray(point_cloud, np.float32)
    weights = np.asarray(weights, np.float32)

    nc = _get_nc(1)
    in_maps, overflow = _prep_inputs(current_map, point_cloud, weights)
    res = run_bass_kernel_spmd(nc, in_maps, core_ids=list(range(N_CORES)))
    LAST_EXEC_NS = res.exec_time_ns

    out = np.empty((GX, GY, GZ, NC), np.float32)
    for c in range(N_CORES):
        blk = res.results[c]["out_blk"].astype(np.float32)  # [32, 128, 1344]
        out[XS * c : XS * (c + 1)] = (
            blk.reshape(XS, 4, 32, GY // 4, NC)
            .transpose(0, 3, 1, 2, 4)
            .reshape(XS, GY, GZ, NC)
        )
    _apply_overflow(out, overflow, weights)
    return out
